# revision 1
# baseline (speedup 1.0000x reference)
"""CrossAttention Trainium2 kernel (8 NeuronCores, SPMD).

Problem: x [4,256,64,64], context [4,512,32,32], 8 heads x 64 dim,
q = Wq@x, k = Wk@ctx, v = Wv@ctx, attn = softmax(q^T k / 8), out = Wo@(v attn^T) + bo.

Sharding: fully data-parallel over (batch, query-spatial-half) -> 8 shards.
Each core computes K/V for its batch (duplicated per pair) and attention +
output projection for its 2048 query positions. Zero collectives.

Per-core pipeline (matmuls in float32r = full-rate):
  simT[j,i] = k^T q per head-pair, two K=64 matmuls packed onto PE row
  groups (tile_position); exp on ScalarE straight out of PSUM; AV
  accumulation with M=65 (64 v-channels + a ones column giving the softmax
  denominator for free); the AV PSUM banks are drained by fast DVE copies so
  PE never waits on the normalization chain (reciprocal + DRAM-bounce
  partition-broadcast + multiply), which happens off the critical path; the
  output projection for i-tile t is emitted during i-tile t+1.

ScalarE (exp: 16.8M elements/core) is the roofline engine; sims are emitted
one slab ahead of AV so PE (in-order) never blocks on ScalarE.
"""
import os
import sys
import numpy as np

for _p in ("/opt/trn_rl_repo", "/root/.axon_site/_ro/trn_rl_repo"):
    if os.path.isdir(_p) and _p not in sys.path:
        sys.path.insert(0, _p)

import concourse.bass as bass
import concourse.mybir as mybir
from concourse.tile import TileContext
from concourse.bass_utils import run_bass_kernel_spmd

F32 = mybir.dt.float32
F32R = mybir.dt.float32r
F16 = mybir.dt.float16
EXP = mybir.ActivationFunctionType.Exp

B, H, D = 4, 8, 64
EQ, EK = 256, 512          # x channels, ctx channels
NQ, NK = 2048, 1024        # per-core query positions, kv positions
OC = 256                   # output channels
SCALE = D ** -0.5
IT, JT = NQ // 512, NK // 128   # 4 i-tiles of 512, 8 j-tiles of 128


def _split_excess_waits(nc, max_waits=1):
    """This walrus build rejects instructions carrying >max_waits sem waits;
    move the extras onto standalone nops just before (same engine, in-order,
    so semantics are unchanged)."""
    n_new = 0
    for f in nc.m.functions:
        for bb in f.blocks:
            insts = list(bb.instructions)
            out = []
            changed = False
            for inst in insts:
                si = inst.sync_info
                if si is not None and si.on_wait and len(si.on_wait) > max_waits:
                    waits = list(si.on_wait)
                    for w in waits[:-max_waits]:
                        nop = mybir.InstNoOp(
                            name=f"I-splitw-{n_new}",
                            sync_info=mybir.SyncInfo(on_wait=[w], on_update=[]),
                        )
                        nop.engine = inst.engine
                        n_new += 1
                        out.append(nop)
                        nc.register_instruction(nop, overwrite=True)
                    si.on_wait = waits[-max_waits:]
                    inst.sync_info = si
                    changed = True
                out.append(inst)
            if changed:
                bb.instructions.clear()
                bb.instructions.extend(out)
    return n_new


def _build():
    nc = bass.Bass()
    x_s = nc.declare_dram_parameter("x_s", [EQ, NQ], F16, isOutput=False)
    ctx_s = nc.declare_dram_parameter("ctx_s", [EK, NK], F16, isOutput=False)
    WqT = nc.declare_dram_parameter("WqT", [EQ, 512], F16, isOutput=False)
    WkT = nc.declare_dram_parameter("WkT", [EK, 512], F16, isOutput=False)
    WvT = nc.declare_dram_parameter("WvT", [EK, 512], F16, isOutput=False)
    WoT = nc.declare_dram_parameter("WoT", [512, OC], F16, isOutput=False)
    bo = nc.declare_dram_parameter("bo", [OC], F32, isOutput=False)
    y = nc.declare_dram_parameter("y", [OC, NQ], F32, isOutput=True)

    sscratch = nc.dram_tensor("sscratch", [IT, 4, 2, 512], F32)
    sscratch2 = nc.dram_tensor("sscratch2", [IT, 4096], F32)

    with TileContext(nc) as tc:
        with (
            tc.tile_pool(name="consts", bufs=1) as cp,
            tc.tile_pool(name="qkv", bufs=1) as qp,
            tc.tile_pool(name="exps", bufs=4) as ep,
            tc.tile_pool(name="avrp", bufs=4) as avrp,
            tc.tile_pool(name="avnp", bufs=8) as avnp,
            tc.tile_pool(name="work", bufs=3) as wp,
            tc.tile_pool(name="slab", bufs=2, space="PSUM") as slabp,
            tc.tile_pool(name="avp", bufs=1, space="PSUM") as avp,
            tc.tile_pool(name="yp", bufs=2, space="PSUM") as yp,
        ):
            # ---- const / weight / input loads (K,V deps first) ----
            wkt = cp.tile([128, 4 * 512], F16, tag="wkt")
            wvt = cp.tile([128, 4 * 512], F16, tag="wvt")
            ctx_sb = cp.tile([128, 4 * NK], F16, tag="ctx_sb")
            wqt = cp.tile([128, 2 * 512], F16, tag="wqt")
            x_sb = cp.tile([128, 2 * NQ], F16, tag="x_sb")
            wot = cp.tile([128, 4 * OC], F16, tag="wot")
            bo_col = cp.tile([128, 2], F32, tag="bo_col")
            for ec in range(4):
                nc.sync.dma_start(out=ctx_sb[:, ec * NK:(ec + 1) * NK],
                                  in_=ctx_s[ec * 128:(ec + 1) * 128, :])
                nc.sync.dma_start(out=wkt[:, ec * 512:(ec + 1) * 512],
                                  in_=WkT[ec * 128:(ec + 1) * 128, :])
                nc.sync.dma_start(out=wvt[:, ec * 512:(ec + 1) * 512],
                                  in_=WvT[ec * 128:(ec + 1) * 128, :])
            for ec in range(2):
                nc.sync.dma_start(out=wqt[:, ec * 512:(ec + 1) * 512],
                                  in_=WqT[ec * 128:(ec + 1) * 128, :])
                nc.sync.dma_start(out=x_sb[:, ec * NQ:(ec + 1) * NQ],
                                  in_=x_s[ec * 128:(ec + 1) * 128, :])
            for ec in range(4):
                nc.sync.dma_start(out=wot[:, ec * OC:(ec + 1) * OC],
                                  in_=WoT[ec * 128:(ec + 1) * 128, :])
            for ob in range(2):
                nc.sync.dma_start(out=bo_col[:, ob:ob + 1],
                                  in_=bo[ob * 128:(ob + 1) * 128])

            # persistent activations
            q_sb = qp.tile([128, 4 * NQ], F16, tag="q_sb")      # [hp, i]
            k_sb = qp.tile([128, 4 * NK], F16, tag="k_sb")      # [hp, j]
            vt_sb = qp.tile([128, JT * 520], F16, tag="vt_sb")  # [jt, h*65 + c]

            # ones columns of vt (col 64 of each 65-block)
            vt_4d = vt_sb.rearrange("p (j h c) -> p j h c", j=JT, h=H)
            ones_f32 = cp.tile([128, JT * H], F32, tag="ones_f32")
            nc.vector.memset(ones_f32, 1.0)
            nc.vector.tensor_copy(
                vt_4d[:, :, :, 64:65],
                ones_f32.rearrange("p (j h) -> p j h", j=JT).unsqueeze(-1))

            # ---- K / VT projections (PSUM via yp pool; no extra banks) ----
            for hp in range(4):
                for ntile in range(NK // 512):
                    pk = yp.tile([128, 512], F32, tag="yps")
                    for ec in range(4):
                        nc.tensor.matmul(
                            pk,
                            lhsT=wkt[:, ec * 512 + hp * 128: ec * 512 + (hp + 1) * 128],
                            rhs=ctx_sb[:, ec * NK + ntile * 512: ec * NK + (ntile + 1) * 512],
                            start=(ec == 0), stop=(ec == 3))
                    nc.vector.tensor_copy(
                        k_sb[:, hp * NK + ntile * 512: hp * NK + (ntile + 1) * 512], pk)
            for jt in range(JT):
                pv = yp.tile([128, 512], F32, tag="yps")
                for ec in range(4):
                    nc.tensor.matmul(
                        pv,
                        lhsT=ctx_sb[:, ec * NK + jt * 128: ec * NK + (jt + 1) * 128],
                        rhs=wvt[:, ec * 512:(ec + 1) * 512],
                        start=(ec == 0), stop=(ec == 3))
                vt_t = vt_sb[:, jt * 520:(jt + 1) * 520].rearrange(
                    "p (h c) -> p h c", h=H)[:, :, 0:64]
                nc.vector.tensor_copy(vt_t, pv.rearrange("p (h c) -> p h c", c=64))

            def q_proj(hp):
                for ntile in range(IT):
                    pq = yp.tile([128, 512], F32, tag="yps")
                    for ec in range(2):
                        nc.tensor.matmul(
                            pq,
                            lhsT=wqt[:, ec * 512 + hp * 128: ec * 512 + (hp + 1) * 128],
                            rhs=x_sb[:, ec * NQ + ntile * 512: ec * NQ + (ntile + 1) * 512],
                            start=(ec == 0), stop=(ec == 1))
                    nc.vector.tensor_copy(
                        q_sb[:, hp * NQ + ntile * 512: hp * NQ + (ntile + 1) * 512], pq)

            q_proj(0)

            def sim_emit(hp, it, jt):
                slab = slabp.tile([128, 1024], F32, tag="slab")
                ks = slice(hp * NK + jt * 128, hp * NK + (jt + 1) * 128)
                qs = slice(hp * NQ + it * 512, hp * NQ + (it + 1) * 512)
                nc.tensor.matmul(
                    slab[:, 0:512], lhsT=k_sb[0:64, ks], rhs=q_sb[0:64, qs],
                    start=True, stop=True, tile_position=(0, 0))
                nc.tensor.matmul(
                    slab[:, 512:1024], lhsT=k_sb[64:128, ks], rhs=q_sb[64:128, qs],
                    start=True, stop=True, tile_position=(64, 0))
                return slab

            def oproj_emit(avn_tiles, it):
                for ob in range(2):
                    yps = yp.tile([128, 512], F32, tag="yps")
                    for cc in range(4):
                        nc.tensor.matmul(
                            yps,
                            lhsT=wot[:, cc * OC + ob * 128: cc * OC + (ob + 1) * 128],
                            rhs=avn_tiles[cc],
                            start=(cc == 0), stop=(cc == 3))
                    ysb = wp.tile([128, 512], F32, tag="ysb")
                    nc.vector.tensor_scalar_add(ysb, yps, bo_col[:, ob:ob + 1])
                    nc.sync.dma_start(
                        out=y[ob * 128:(ob + 1) * 128, it * 512:(it + 1) * 512],
                        in_=ysb)

            # ---- attention ----
            prev = None   # (avn_tiles, it) pending output projection
            for it in range(IT):
                pend = []  # per-hp (avr0, avr1, sbc) awaiting normalization
                for hp in range(4):
                    av0 = avp.tile([65, 512], F32, tag="av0")
                    av1 = avp.tile([65, 512], F32, tag="av1")
                    slab = sim_emit(hp, it, 0)
                    for jt in range(JT):
                        nslab = sim_emit(hp, it, jt + 1) if jt + 1 < JT else None
                        exps = ep.tile([128, 1024], F16, tag="exps")
                        nc.scalar.activation(exps, slab, EXP, bias=0.0, scale=SCALE)
                        nc.tensor.matmul(
                            av0,
                            lhsT=vt_sb[:, jt * 520 + (2 * hp) * 65: jt * 520 + (2 * hp) * 65 + 65],
                            rhs=exps[:, 0:512],
                            start=(jt == 0), stop=(jt == JT - 1))
                        nc.tensor.matmul(
                            av1,
                            lhsT=vt_sb[:, jt * 520 + (2 * hp + 1) * 65: jt * 520 + (2 * hp + 1) * 65 + 65],
                            rhs=exps[:, 512:1024],
                            start=(jt == 0), stop=(jt == JT - 1))
                        slab = nslab
                    # drain AV PSUM banks quickly (frees them for hp+1);
                    # row 64 carries the softmax denominator
                    avr0 = avrp.tile([65, 512], F32, tag="avr0")
                    avr1 = avrp.tile([65, 512], F32, tag="avr1")
                    nc.vector.tensor_copy(avr0, av0)
                    nc.vector.tensor_copy(avr1, av1)
                    nc.sync.dma_start(out=sscratch[it, hp, 0], in_=avr0[64:65, :])
                    nc.sync.dma_start(out=sscratch[it, hp, 1], in_=avr1[64:65, :])
                    pend.append((avr0, avr1))
                    if hp == 0:
                        if it == 0:
                            for nhp in range(1, 4):
                                q_proj(nhp)
                        if prev is not None:
                            oproj_emit(*prev)
                            prev = None
                # normalization (off the PE critical path):
                # one lane-parallel reciprocal over all 8 denominator rows
                stile = wp.tile([128, 32], F32, tag="stile")
                nc.sync.dma_start(
                    out=stile,
                    in_=bass.AP(tensor=sscratch, offset=it * 4096,
                                ap=[[32, 128], [1, 32]]))
                stile_r = wp.tile([128, 32], F32, tag="stile_r")
                nc.vector.reciprocal(stile_r, stile)
                nc.sync.dma_start(
                    out=sscratch2[it].rearrange("(p f) -> p f", p=128),
                    in_=stile_r)
                avn_tiles = []
                for hp, (avr0, avr1) in enumerate(pend):
                    base = it * 4096 + hp * 1024
                    sbc_a = wp.tile([64, 512], F32, tag="sbc_a")
                    sbc_b = wp.tile([64, 512], F32, tag="sbc_b")
                    nc.sync.dma_start(
                        out=sbc_a,
                        in_=bass.AP(tensor=sscratch2, offset=base,
                                    ap=[[0, 64], [1, 512]]))
                    nc.sync.dma_start(
                        out=sbc_b,
                        in_=bass.AP(tensor=sscratch2, offset=base + 512,
                                    ap=[[0, 64], [1, 512]]))
                    avn = avnp.tile([128, 512], F16, tag="avn")
                    nc.vector.tensor_mul(avn[0:64, :], avr0[0:64, :], sbc_a)
                    nc.vector.tensor_mul(avn[64:128, :], avr1[0:64, :], sbc_b)
                    avn_tiles.append(avn)
                prev = (avn_tiles, it)
            oproj_emit(*prev)

    _split_excess_waits(nc)
    return nc


_CACHED = None


def kernel(x, context, Wq, Wk, Wv, Wo, bo):
    global _CACHED
    if _CACHED is None:
        _CACHED = _build()
    nc = _CACHED

    x = np.asarray(x, dtype=np.float32)
    context = np.asarray(context, dtype=np.float32)
    xf = x.reshape(B, EQ, 64 * 64)
    cf = context.reshape(B, EK, 32 * 32)
    WqT = np.ascontiguousarray(np.asarray(Wq, np.float32).T.astype(np.float16))
    WkT = np.ascontiguousarray(np.asarray(Wk, np.float32).T.astype(np.float16))
    WvT = np.ascontiguousarray(np.asarray(Wv, np.float32).T.astype(np.float16))
    WoT = np.ascontiguousarray(np.asarray(Wo, np.float32).T.astype(np.float16))
    bo = np.ascontiguousarray(np.asarray(bo, np.float32))

    in_maps = []
    for core in range(8):
        b, half = core // 2, core % 2
        in_maps.append({
            "x_s": np.ascontiguousarray(xf[b, :, half * NQ:(half + 1) * NQ]).astype(np.float16),
            "ctx_s": np.ascontiguousarray(cf[b]).astype(np.float16),
            "WqT": WqT, "WkT": WkT, "WvT": WvT, "WoT": WoT, "bo": bo,
        })

    res = run_bass_kernel_spmd(nc, in_maps, list(range(8)))
    kernel.last_results = res

    out = np.empty((B, OC, 64 * 64), dtype=np.float32)
    for core in range(8):
        b, half = core // 2, core % 2
        out[b, :, half * NQ:(half + 1) * NQ] = res.results[core]["y"]
    return out.reshape(B, OC, 64, 64)



# revision 6
# speedup vs baseline: 1.1320x; 1.1320x over previous
"""CrossAttention Trainium2 kernel (8 NeuronCores, SPMD), v2.

Problem: x [4,256,64,64], context [4,512,32,32], 8 heads x 64 dim,
q = Wq@x, k = Wk@ctx, v = Wv@ctx, attn = softmax(q^T k / 8), out = Wo@(v attn^T) + bo.

Sharding: fully data-parallel over (batch, query-spatial-half) -> 8 shards.
Zero collectives.  ScalarE exp (16.8M elem/core) is the roofline engine.

v2 structure vs v1:
  - 4 coalesced input DMAs (host-packed blobs) instead of 22.
  - PE warm-up matmuls during the input DMA so HAM reaches 8/8 early.
  - Minimal critical path to first exp: K(hp0) + Q(hp0,it0) only; V/K/Q
    projections interleaved into the first i-tile's attention blocks.
  - Per-(it,hp) normalization chains (den row -> SBUF-SBUF gather DMA ->
    lane-parallel reciprocal -> DRAM scatter -> partition-broadcast DMA),
    software-pipelined 4 blocks deep; f16 throughout.
  - Output projection for i-tile t emitted inside block (t+1, hp3).
  - Optional DVE "fast-exp" offload: exp(x) ~= bitcast_f16(i16(x*A + B)),
    computed by one tensor_scalar per offloaded slab to relieve ScalarE.
"""
import os
import sys
import numpy as np

for _p in ("/opt/trn_rl_repo", "/root/.axon_site/_ro/trn_rl_repo"):
    if os.path.isdir(_p) and _p not in sys.path:
        sys.path.insert(0, _p)

import concourse.bass as bass
import concourse.mybir as mybir
from concourse.tile import TileContext
from concourse.bass_utils import run_bass_kernel_spmd

F32 = mybir.dt.float32
F16 = mybir.dt.float16
I16 = mybir.dt.int16
EXP = mybir.ActivationFunctionType.Exp

B, H, D = 4, 8, 64
EQ, EK = 256, 512
NQ, NK = 2048, 1024
OC = 256
SCALE = D ** -0.5
IT, JT = NQ // 512, NK // 128

# DVE fast-exp offload: which jt slabs go to VectorE instead of ScalarE.
OFFLOAD = tuple(int(x) for x in os.environ.get("KOFFLOAD", "").split(",") if x != "")
FEXP_A = float(np.log2(np.e) * 1024.0 * SCALE)
FEXP_B = 15360.0 - 36.0


def _split_excess_waits(nc, max_waits=1):
    """This walrus build rejects instructions carrying >max_waits sem waits;
    move the extras onto standalone nops just before (same engine, in-order,
    so semantics are unchanged)."""
    n_new = 0
    for f in nc.m.functions:
        for bb in f.blocks:
            insts = list(bb.instructions)
            out = []
            changed = False
            for inst in insts:
                si = inst.sync_info
                if si is not None and si.on_wait and len(si.on_wait) > max_waits:
                    waits = list(si.on_wait)
                    for w in waits[:-max_waits]:
                        nop = mybir.InstNoOp(
                            name=f"I-splitw-{n_new}",
                            sync_info=mybir.SyncInfo(on_wait=[w], on_update=[]),
                        )
                        nop.engine = inst.engine
                        n_new += 1
                        out.append(nop)
                        nc.register_instruction(nop, overwrite=True)
                    si.on_wait = waits[-max_waits:]
                    inst.sync_info = si
                    changed = True
                out.append(inst)
            if changed:
                bb.instructions.clear()
                bb.instructions.extend(out)
    return n_new


def _build():
    nc = bass.Bass()
    # host-packed blobs (f16):
    #  blob1 [128, 6144]: ctx (4 ec-chunks of 1024) + WkT (4 ec-chunks of 512)
    #  blob2 [128, 5120]: x   (2 ec-chunks of 2048) + WqT (2 ec-chunks of 512)
    #  blob3 [128, 3072]: WvT (4 ec-chunks of 512)  + WoT (4 cc-chunks of 256)
    blob1 = nc.declare_dram_parameter("blob1", [128, 6144], F16, isOutput=False)
    blob2 = nc.declare_dram_parameter("blob2", [128, 5120], F16, isOutput=False)
    blob3 = nc.declare_dram_parameter("blob3", [128, 3072], F16, isOutput=False)
    bo2 = nc.declare_dram_parameter("bo2", [128, 2], F32, isOutput=False)
    y = nc.declare_dram_parameter("y", [OC, NQ], F32, isOutput=True)

    sscratch2 = nc.dram_tensor("sscratch2", [IT * 4 * 1024], F16)

    with TileContext(nc) as tc:
        with (
            tc.tile_pool(name="consts", bufs=1) as cp,
            tc.tile_pool(name="qkv", bufs=1) as qp,
            tc.tile_pool(name="exps", bufs=4) as ep,
            tc.tile_pool(name="avrp", bufs=8) as avrp,
            tc.tile_pool(name="bcastp", bufs=6) as bcp,
            tc.tile_pool(name="avnp", bufs=6) as avnp,
            tc.tile_pool(name="work", bufs=4) as wp,
            tc.tile_pool(name="slab", bufs=2, space="PSUM") as slabp,
            tc.tile_pool(name="avp", bufs=1, space="PSUM") as avp,
            tc.tile_pool(name="yp", bufs=2, space="PSUM") as yp,
        ):
            big1 = cp.tile([128, 6144], F16, tag="big1")
            big2 = cp.tile([128, 5120], F16, tag="big2")
            big3 = cp.tile([128, 3072], F16, tag="big3")
            bo_col = cp.tile([128, 2], F32, tag="bo_col")
            dummy = cp.tile([128, 512], F16, tag="dummy")
            ones_f32 = cp.tile([128, JT * H], F32, tag="ones_f32")

            # PE warm-up: runs during the input DMAs (no data deps), flips
            # HAM to 8/8 before the first real matmul.
            nc.vector.memset(dummy, 0.0)
            nc.vector.memset(ones_f32, 1.0)
            warm = yp.tile([128, 512], F32, tag="yps")
            for i in range(18):
                nc.tensor.matmul(warm, lhsT=dummy[:, 0:128], rhs=dummy,
                                 start=(i == 0), stop=(i == 17))

            nc.sync.dma_start(out=big1, in_=blob1[:, :])
            nc.sync.dma_start(out=big2, in_=blob2[:, :])
            nc.sync.dma_start(out=big3, in_=blob3[:, :])
            nc.sync.dma_start(out=bo_col, in_=bo2[:, :])

            def ctx_sb(ec, lo, hi):
                return big1[:, ec * 1024 + lo: ec * 1024 + hi]

            def wkt(ec, lo, hi):
                return big1[:, 4096 + ec * 512 + lo: 4096 + ec * 512 + hi]

            def x_sb(ec, lo, hi):
                return big2[:, ec * 2048 + lo: ec * 2048 + hi]

            def wqt(ec, lo, hi):
                return big2[:, 4096 + ec * 512 + lo: 4096 + ec * 512 + hi]

            def wvt(ec):
                return big3[:, ec * 512:(ec + 1) * 512]

            def wot(cc, lo, hi):
                return big3[:, 2048 + cc * 256 + lo: 2048 + cc * 256 + hi]

            # persistent activations
            q_sb = qp.tile([128, 4 * NQ], F16, tag="q_sb")      # [hp, i]
            k_sb = qp.tile([128, 4 * NK], F16, tag="k_sb")      # [hp, j]
            vt_sb = qp.tile([128, JT * 520], F16, tag="vt_sb")  # [jt, h*65 + c]

            # ones columns of vt (col 64 of each 65-block) -> softmax denom
            vt_4d = vt_sb.rearrange("p (j h c) -> p j h c", j=JT, h=H)
            nc.vector.tensor_copy(
                vt_4d[:, :, :, 64:65],
                ones_f32.rearrange("p (j h) -> p j h", j=JT).unsqueeze(-1))

            def k_proj(hp):
                for ntile in range(NK // 512):
                    pk = yp.tile([128, 512], F32, tag="yps")
                    for ec in range(4):
                        nc.tensor.matmul(
                            pk,
                            lhsT=wkt(ec, hp * 128, (hp + 1) * 128),
                            rhs=ctx_sb(ec, ntile * 512, (ntile + 1) * 512),
                            start=(ec == 0), stop=(ec == 3))
                    nc.vector.tensor_copy(
                        k_sb[:, hp * NK + ntile * 512: hp * NK + (ntile + 1) * 512], pk)

            def q_proj_tile(hp, ntile):
                pq = yp.tile([128, 512], F32, tag="yps")
                for ec in range(2):
                    nc.tensor.matmul(
                        pq,
                        lhsT=wqt(ec, hp * 128, (hp + 1) * 128),
                        rhs=x_sb(ec, ntile * 512, (ntile + 1) * 512),
                        start=(ec == 0), stop=(ec == 1))
                nc.vector.tensor_copy(
                    q_sb[:, hp * NQ + ntile * 512: hp * NQ + (ntile + 1) * 512], pq)

            def v_proj(jt):
                pv = yp.tile([128, 512], F32, tag="yps")
                for ec in range(4):
                    nc.tensor.matmul(
                        pv,
                        lhsT=ctx_sb(ec, jt * 128, (jt + 1) * 128),
                        rhs=wvt(ec),
                        start=(ec == 0), stop=(ec == 3))
                vt_t = vt_sb[:, jt * 520:(jt + 1) * 520].rearrange(
                    "p (h c) -> p h c", h=H)[:, :, 0:64]
                nc.vector.tensor_copy(vt_t, pv.rearrange("p (h c) -> p h c", c=64))

            k_proj(0)
            q_proj_tile(0, 0)

            def sim_emit(hp, it, jt):
                slab = slabp.tile([128, 1024], F32, tag="slab")
                ks = slice(hp * NK + jt * 128, hp * NK + (jt + 1) * 128)
                qs = slice(hp * NQ + it * 512, hp * NQ + (it + 1) * 512)
                nc.tensor.matmul(
                    slab[:, 0:512], lhsT=k_sb[0:64, ks], rhs=q_sb[0:64, qs],
                    start=True, stop=True, tile_position=(0, 0))
                nc.tensor.matmul(
                    slab[:, 512:1024], lhsT=k_sb[64:128, ks], rhs=q_sb[64:128, qs],
                    start=True, stop=True, tile_position=(64, 0))
                return slab

            avr_tiles = {}    # (it, hp) -> avr [65, 1024] f16
            bcast_tiles = {}  # (it, hp) -> bcast [64, 1024] f16
            avn_tiles = {}    # (it, hp) -> avn [128, 512] f16

            def norm_mul(it, hp):
                avr = avr_tiles.pop((it, hp))
                bcast = bcast_tiles.pop((it, hp))
                avn = avnp.tile([128, 512], F16, tag="avn")
                nc.vector.tensor_mul(avn[0:64, :], avr[0:64, 0:512], bcast[:, 0:512])
                nc.vector.tensor_mul(avn[64:128, :], avr[0:64, 512:1024], bcast[:, 512:1024])
                avn_tiles[(it, hp)] = avn

            def oproj(it):
                tiles = [avn_tiles.pop((it, cc)) for cc in range(4)]
                for ob in range(2):
                    yps = yp.tile([128, 512], F32, tag="yps")
                    for cc in range(4):
                        nc.tensor.matmul(
                            yps,
                            lhsT=wot(cc, ob * 128, (ob + 1) * 128),
                            rhs=tiles[cc],
                            start=(cc == 0), stop=(cc == 3))
                    ysb = wp.tile([128, 512], F32, tag="ysb")
                    nc.vector.tensor_scalar_add(ysb, yps, bo_col[:, ob:ob + 1])
                    nc.sync.dma_start(
                        out=y[ob * 128:(ob + 1) * 128, it * 512:(it + 1) * 512],
                        in_=ysb)

            # ---- attention blocks ----
            for it in range(IT):
                for hp in range(4):
                    first_block = (it == 0 and hp == 0)
                    if it > 0:
                        norm_mul(it - 1, hp)
                        if hp == 3:
                            oproj(it - 1)
                    if it < 3:
                        q_proj_tile(hp, it + 1)

                    av0 = avp.tile([65, 512], F32, tag="av0")
                    av1 = avp.tile([65, 512], F32, tag="av1")
                    slab = sim_emit(hp, it, 0)
                    for jt in range(JT):
                        nslab = sim_emit(hp, it, jt + 1) if jt + 1 < JT else None
                        if jt in OFFLOAD:
                            fx = ep.tile([128, 1024], I16, tag="fexp")
                            nc.vector.tensor_scalar(
                                fx, slab, FEXP_A, FEXP_B,
                                mybir.AluOpType.mult, mybir.AluOpType.add)
                            exps = fx.bitcast(F16)
                        else:
                            exps = ep.tile([128, 1024], F16, tag="exps")
                            nc.scalar.activation(exps, slab, EXP, bias=0.0, scale=SCALE)
                        if first_block:
                            v_proj(jt)
                        nc.tensor.matmul(
                            av0,
                            lhsT=vt_sb[:, jt * 520 + (2 * hp) * 65: jt * 520 + (2 * hp) * 65 + 65],
                            rhs=exps[:, 0:512],
                            start=(jt == 0), stop=(jt == JT - 1))
                        nc.tensor.matmul(
                            av1,
                            lhsT=vt_sb[:, jt * 520 + (2 * hp + 1) * 65: jt * 520 + (2 * hp + 1) * 65 + 65],
                            rhs=exps[:, 512:1024],
                            start=(jt == 0), stop=(jt == JT - 1))
                        slab = nslab

                    # drain AV banks (f16) + per-(it,hp) normalization chain
                    avr = avrp.tile([65, 1024], F16, tag="avr")
                    nc.vector.tensor_copy(avr[:, 0:512], av0)
                    nc.vector.tensor_copy(avr[:, 512:1024], av1)
                    avr_tiles[(it, hp)] = avr
                    stile = wp.tile([128, 8], F16, tag="stile")
                    nc.sync.dma_start(out=stile, in_=avr[64:65, 0:1024])
                    stile_r = wp.tile([128, 8], F16, tag="stile_r")
                    with nc.allow_low_precision(reason="f16 softmax denominators"):
                        nc.vector.reciprocal(stile_r, stile)
                    base = (it * 4 + hp) * 1024
                    nc.sync.dma_start(
                        out=bass.AP(tensor=sscratch2, offset=base, ap=[[1, 1024]]),
                        in_=stile_r)
                    bcast = bcp.tile([64, 1024], F16, tag="bcast")
                    nc.sync.dma_start(
                        out=bcast,
                        in_=bass.AP(tensor=sscratch2, offset=base,
                                    ap=[[0, 64], [1, 1024]]))
                    bcast_tiles[(it, hp)] = bcast

                    if it == 0 and hp < 3:
                        k_proj(hp + 1)
                        q_proj_tile(hp + 1, 0)

            for hp in range(4):
                norm_mul(3, hp)
            oproj(3)

    _split_excess_waits(nc)
    return nc


_CACHED = None


def kernel(x, context, Wq, Wk, Wv, Wo, bo):
    global _CACHED
    if _CACHED is None:
        _CACHED = _build()
    nc = _CACHED

    x = np.asarray(x, dtype=np.float32)
    context = np.asarray(context, dtype=np.float32)
    xf = x.reshape(B, EQ, 64 * 64)
    cf = context.reshape(B, EK, 32 * 32)
    WqT = np.asarray(Wq, np.float32).T.astype(np.float16)   # [EQ, 512]
    WkT = np.asarray(Wk, np.float32).T.astype(np.float16)   # [EK, 512]
    WvT = np.asarray(Wv, np.float32).T.astype(np.float16)   # [EK, 512]
    WoT = np.asarray(Wo, np.float32).T.astype(np.float16)   # [512, OC]
    bo = np.asarray(bo, np.float32)

    # blob1 per batch: ctx chunks + WkT chunks
    def chunks(a, n):
        # [n*128, F] -> [128, n*F] with chunk-major free layout
        return a.reshape(n, 128, -1).transpose(1, 0, 2).reshape(128, -1)

    blob1s = []
    for b in range(B):
        blob1s.append(np.ascontiguousarray(np.concatenate(
            [chunks(cf[b].astype(np.float16), 4), chunks(WkT, 4)], axis=1)))
    # blob2 per core: x half chunks + WqT chunks
    q16 = chunks(WqT, 2)
    blob3 = np.ascontiguousarray(
        np.concatenate([chunks(WvT, 4), chunks(WoT, 4)], axis=1))
    bo2 = np.ascontiguousarray(bo.reshape(2, 128).T)

    in_maps = []
    for core in range(8):
        b, half = core // 2, core % 2
        x16 = chunks(xf[b, :, half * NQ:(half + 1) * NQ].astype(np.float16), 2)
        blob2 = np.ascontiguousarray(np.concatenate([x16, q16], axis=1))
        in_maps.append({
            "blob1": blob1s[b], "blob2": blob2, "blob3": blob3, "bo2": bo2,
        })

    res = run_bass_kernel_spmd(nc, in_maps, list(range(8)))
    kernel.last_results = res

    out = np.empty((B, OC, 64 * 64), dtype=np.float32)
    for core in range(8):
        b, half = core // 2, core % 2
        out[b, :, half * NQ:(half + 1) * NQ] = res.results[core]["y"]
    return out.reshape(B, OC, 64, 64)


# revision 7
# speedup vs baseline: 1.1572x; 1.0223x over previous
"""CrossAttention Trainium2 kernel (8 NeuronCores, SPMD), v2.

Problem: x [4,256,64,64], context [4,512,32,32], 8 heads x 64 dim,
q = Wq@x, k = Wk@ctx, v = Wv@ctx, attn = softmax(q^T k / 8), out = Wo@(v attn^T) + bo.

Sharding: fully data-parallel over (batch, query-spatial-half) -> 8 shards.
Zero collectives.  ScalarE exp (16.8M elem/core) is the roofline engine.

v2 structure vs v1:
  - 4 coalesced input DMAs (host-packed blobs) instead of 22.
  - PE warm-up matmuls during the input DMA so HAM reaches 8/8 early.
  - Minimal critical path to first exp: K(hp0) + Q(hp0,it0) only; V/K/Q
    projections interleaved into the first i-tile's attention blocks.
  - Per-(it,hp) normalization chains (den row -> SBUF-SBUF gather DMA ->
    lane-parallel reciprocal -> DRAM scatter -> partition-broadcast DMA),
    software-pipelined 4 blocks deep; f16 throughout.
  - Output projection for i-tile t emitted inside block (t+1, hp3).
  - Optional DVE "fast-exp" offload: exp(x) ~= bitcast_f16(i16(x*A + B)),
    computed by one tensor_scalar per offloaded slab to relieve ScalarE.
"""
import os
import sys
import numpy as np

for _p in ("/opt/trn_rl_repo", "/root/.axon_site/_ro/trn_rl_repo"):
    if os.path.isdir(_p) and _p not in sys.path:
        sys.path.insert(0, _p)

import concourse.bass as bass
import concourse.mybir as mybir
from concourse.tile import TileContext
from concourse.bass_utils import run_bass_kernel_spmd

F32 = mybir.dt.float32
F16 = mybir.dt.float16
I16 = mybir.dt.int16
EXP = mybir.ActivationFunctionType.Exp

B, H, D = 4, 8, 64
EQ, EK = 256, 512
NQ, NK = 2048, 1024
OC = 256
SCALE = D ** -0.5
IT, JT = NQ // 512, NK // 128

# DVE fast-exp offload: which jt slabs go to VectorE instead of ScalarE.
OFFLOAD = tuple(int(x) for x in os.environ.get("KOFFLOAD", "").split(",") if x != "")
FEXP_A = float(np.log2(np.e) * 1024.0 * SCALE)
FEXP_B = 15360.0 - 36.0


def _split_excess_waits(nc, max_waits=1):
    """This walrus build rejects instructions carrying >max_waits sem waits;
    move the extras onto standalone nops just before (same engine, in-order,
    so semantics are unchanged)."""
    n_new = 0
    for f in nc.m.functions:
        for bb in f.blocks:
            insts = list(bb.instructions)
            out = []
            changed = False
            for inst in insts:
                si = inst.sync_info
                if si is not None and si.on_wait and len(si.on_wait) > max_waits:
                    waits = list(si.on_wait)
                    for w in waits[:-max_waits]:
                        nop = mybir.InstNoOp(
                            name=f"I-splitw-{n_new}",
                            sync_info=mybir.SyncInfo(on_wait=[w], on_update=[]),
                        )
                        nop.engine = inst.engine
                        n_new += 1
                        out.append(nop)
                        nc.register_instruction(nop, overwrite=True)
                    si.on_wait = waits[-max_waits:]
                    inst.sync_info = si
                    changed = True
                out.append(inst)
            if changed:
                bb.instructions.clear()
                bb.instructions.extend(out)
    return n_new


def _build():
    nc = bass.Bass()
    # host-packed blobs (f16):
    #  blob1 [128, 6144]: ctx (4 ec-chunks of 1024) + WkT (4 ec-chunks of 512)
    #  blob2 [128, 5120]: x   (2 ec-chunks of 2048) + WqT (2 ec-chunks of 512)
    #  blob3 [128, 3072]: WvT (4 ec-chunks of 512)  + WoT (4 cc-chunks of 256)
    blobA = nc.declare_dram_parameter("blobA", [128, 5888], F16, isOutput=False)
    blobB1 = nc.declare_dram_parameter("blobB1", [128, 4352], F16, isOutput=False)
    blobB2 = nc.declare_dram_parameter("blobB2", [128, 4096], F16, isOutput=False)
    bo2 = nc.declare_dram_parameter("bo2", [128, 2], F32, isOutput=False)
    y = nc.declare_dram_parameter("y", [OC, NQ], F32, isOutput=True)

    sscratch2 = nc.dram_tensor("sscratch2", [IT * 4 * 1024], F16)

    with TileContext(nc) as tc:
        with (
            tc.tile_pool(name="consts", bufs=1) as cp,
            tc.tile_pool(name="qkv", bufs=1) as qp,
            tc.tile_pool(name="exps", bufs=4) as ep,
            tc.tile_pool(name="avrp", bufs=8) as avrp,
            tc.tile_pool(name="bcastp", bufs=6) as bcp,
            tc.tile_pool(name="avnp", bufs=6) as avnp,
            tc.tile_pool(name="work", bufs=4) as wp,
            tc.tile_pool(name="slab", bufs=2, space="PSUM") as slabp,
            tc.tile_pool(name="avp", bufs=1, space="PSUM") as avp,
            tc.tile_pool(name="yp", bufs=2, space="PSUM") as yp,
        ):
            bigA = cp.tile([128, 5888], F16, tag="bigA")
            bigB1 = cp.tile([128, 4352], F16, tag="bigB1")
            bigB2 = cp.tile([128, 4096], F16, tag="bigB2")
            bo_col = cp.tile([128, 2], F32, tag="bo_col")
            dummy = cp.tile([128, 512], F16, tag="dummy")
            ones_f32 = cp.tile([128, JT * H], F32, tag="ones_f32")

            # PE warm-up: runs during the input DMAs (no data deps), flips
            # HAM to 8/8 before the first real matmul.
            nc.vector.memset(dummy, 0.0)
            nc.vector.memset(ones_f32, 1.0)
            warm = yp.tile([128, 512], F32, tag="yps")
            for i in range(18):
                nc.tensor.matmul(warm, lhsT=dummy[:, 0:128], rhs=dummy,
                                 start=(i == 0), stop=(i == 17))

            nc.sync.dma_start(out=bigA, in_=blobA[:, :])
            nc.sync.dma_start(out=bigB1, in_=blobB1[:, :])
            nc.sync.dma_start(out=bigB2, in_=blobB2[:, :])
            nc.sync.dma_start(out=bo_col, in_=bo2[:, :])

            def ctx_sb(ec, lo, hi):
                return bigA[:, ec * 1024 + lo: ec * 1024 + hi]

            def wkt(ec, hp):
                if hp == 0:
                    return bigA[:, 4096 + ec * 128: 4096 + (ec + 1) * 128]
                o = 2048 + ec * 384 + (hp - 1) * 128
                return bigB1[:, o: o + 128]

            def wqt(ec, hp):
                if hp == 0:
                    return bigA[:, 4608 + ec * 128: 4608 + (ec + 1) * 128]
                o = 3584 + ec * 384 + (hp - 1) * 128
                return bigB1[:, o: o + 128]

            def x_sb(ec, it, lo, hi):
                if it == 0:
                    return bigA[:, 4864 + ec * 512 + lo: 4864 + ec * 512 + hi]
                o = ec * 1536 + (it - 1) * 512
                return bigB2[:, o + lo: o + hi]

            def wvt(ec):
                return bigB1[:, ec * 512:(ec + 1) * 512]

            def wot(cc, lo, hi):
                return bigB2[:, 3072 + cc * 256 + lo: 3072 + cc * 256 + hi]

            # persistent activations
            q_sb = qp.tile([128, 4 * NQ], F16, tag="q_sb")      # [hp, i]
            k_sb = qp.tile([128, 4 * NK], F16, tag="k_sb")      # [hp, j]
            vt_sb = qp.tile([128, JT * 520], F16, tag="vt_sb")  # [jt, h*65 + c]

            # ones columns of vt (col 64 of each 65-block) -> softmax denom
            vt_4d = vt_sb.rearrange("p (j h c) -> p j h c", j=JT, h=H)
            nc.vector.tensor_copy(
                vt_4d[:, :, :, 64:65],
                ones_f32.rearrange("p (j h) -> p j h", j=JT).unsqueeze(-1))

            def k_proj(hp):
                for ntile in range(NK // 512):
                    pk = yp.tile([128, 512], F32, tag="yps")
                    for ec in range(4):
                        nc.tensor.matmul(
                            pk,
                            lhsT=wkt(ec, hp),
                            rhs=ctx_sb(ec, ntile * 512, (ntile + 1) * 512),
                            start=(ec == 0), stop=(ec == 3))
                    nc.vector.tensor_copy(
                        k_sb[:, hp * NK + ntile * 512: hp * NK + (ntile + 1) * 512], pk)

            def q_proj_tile(hp, ntile):
                pq = yp.tile([128, 512], F32, tag="yps")
                for ec in range(2):
                    nc.tensor.matmul(
                        pq,
                        lhsT=wqt(ec, hp),
                        rhs=x_sb(ec, ntile, 0, 512),
                        start=(ec == 0), stop=(ec == 1))
                nc.vector.tensor_copy(
                    q_sb[:, hp * NQ + ntile * 512: hp * NQ + (ntile + 1) * 512], pq)

            def v_proj(jt):
                pv = yp.tile([128, 512], F32, tag="yps")
                for ec in range(4):
                    nc.tensor.matmul(
                        pv,
                        lhsT=ctx_sb(ec, jt * 128, (jt + 1) * 128),
                        rhs=wvt(ec),
                        start=(ec == 0), stop=(ec == 3))
                vt_t = vt_sb[:, jt * 520:(jt + 1) * 520].rearrange(
                    "p (h c) -> p h c", h=H)[:, :, 0:64]
                nc.vector.tensor_copy(vt_t, pv.rearrange("p (h c) -> p h c", c=64))

            k_proj(0)
            q_proj_tile(0, 0)

            def sim_emit(hp, it, jt):
                slab = slabp.tile([128, 1024], F32, tag="slab")
                ks = slice(hp * NK + jt * 128, hp * NK + (jt + 1) * 128)
                qs = slice(hp * NQ + it * 512, hp * NQ + (it + 1) * 512)
                nc.tensor.matmul(
                    slab[:, 0:512], lhsT=k_sb[0:64, ks], rhs=q_sb[0:64, qs],
                    start=True, stop=True, tile_position=(0, 0))
                nc.tensor.matmul(
                    slab[:, 512:1024], lhsT=k_sb[64:128, ks], rhs=q_sb[64:128, qs],
                    start=True, stop=True, tile_position=(64, 0))
                return slab

            avr_tiles = {}    # (it, hp) -> avr [65, 1024] f16
            bcast_tiles = {}  # (it, hp) -> bcast [64, 1024] f16
            avn_tiles = {}    # (it, hp) -> avn [128, 512] f16

            def norm_mul(it, hp):
                avr = avr_tiles.pop((it, hp))
                bcast = bcast_tiles.pop((it, hp))
                avn = avnp.tile([128, 512], F16, tag="avn")
                nc.vector.tensor_mul(avn[0:64, :], avr[0:64, 0:512], bcast[:, 0:512])
                nc.vector.tensor_mul(avn[64:128, :], avr[0:64, 512:1024], bcast[:, 512:1024])
                avn_tiles[(it, hp)] = avn

            def oproj(it):
                tiles = [avn_tiles.pop((it, cc)) for cc in range(4)]
                for ob in range(2):
                    yps = yp.tile([128, 512], F32, tag="yps")
                    for cc in range(4):
                        nc.tensor.matmul(
                            yps,
                            lhsT=wot(cc, ob * 128, (ob + 1) * 128),
                            rhs=tiles[cc],
                            start=(cc == 0), stop=(cc == 3))
                    ysb = wp.tile([128, 512], F32, tag="ysb")
                    nc.vector.tensor_scalar_add(ysb, yps, bo_col[:, ob:ob + 1])
                    nc.sync.dma_start(
                        out=y[ob * 128:(ob + 1) * 128, it * 512:(it + 1) * 512],
                        in_=ysb)

            # ---- attention blocks ----
            for it in range(IT):
                for hp in range(4):
                    first_block = (it == 0 and hp == 0)
                    if it > 0:
                        norm_mul(it - 1, hp)
                        if hp == 3:
                            oproj(it - 1)
                    if it == 3 and hp >= 2:
                        norm_mul(3, hp - 2)

                    av0 = avp.tile([65, 512], F32, tag="av0")
                    av1 = avp.tile([65, 512], F32, tag="av1")
                    slab = sim_emit(hp, it, 0)
                    for jt in range(JT):
                        nslab = sim_emit(hp, it, jt + 1) if jt + 1 < JT else None
                        if jt in OFFLOAD:
                            fx = ep.tile([128, 1024], I16, tag="fexp")
                            nc.vector.tensor_scalar(
                                fx, slab, FEXP_A, FEXP_B,
                                mybir.AluOpType.mult, mybir.AluOpType.add)
                            exps = fx.bitcast(F16)
                        else:
                            exps = ep.tile([128, 1024], F16, tag="exps")
                            nc.scalar.activation(exps, slab, EXP, bias=0.0, scale=SCALE)
                        if first_block:
                            v_proj(jt)
                        nc.tensor.matmul(
                            av0,
                            lhsT=vt_sb[:, jt * 520 + (2 * hp) * 65: jt * 520 + (2 * hp) * 65 + 65],
                            rhs=exps[:, 0:512],
                            start=(jt == 0), stop=(jt == JT - 1))
                        nc.tensor.matmul(
                            av1,
                            lhsT=vt_sb[:, jt * 520 + (2 * hp + 1) * 65: jt * 520 + (2 * hp + 1) * 65 + 65],
                            rhs=exps[:, 512:1024],
                            start=(jt == 0), stop=(jt == JT - 1))
                        slab = nslab

                    # drain AV banks (f16) + per-(it,hp) normalization chain
                    avr = avrp.tile([65, 1024], F16, tag="avr")
                    nc.vector.tensor_copy(avr[:, 0:512], av0)
                    nc.vector.tensor_copy(avr[:, 512:1024], av1)
                    avr_tiles[(it, hp)] = avr
                    stile = wp.tile([128, 8], F16, tag="stile")
                    nc.sync.dma_start(out=stile, in_=avr[64:65, 0:1024])
                    stile_r = wp.tile([128, 8], F16, tag="stile_r")
                    with nc.allow_low_precision(reason="f16 softmax denominators"):
                        nc.vector.reciprocal(stile_r, stile)
                    base = (it * 4 + hp) * 1024
                    nc.sync.dma_start(
                        out=bass.AP(tensor=sscratch2, offset=base, ap=[[1, 1024]]),
                        in_=stile_r)
                    bcast = bcp.tile([64, 1024], F16, tag="bcast")
                    nc.sync.dma_start(
                        out=bcast,
                        in_=bass.AP(tensor=sscratch2, offset=base,
                                    ap=[[0, 64], [1, 1024]]))
                    bcast_tiles[(it, hp)] = bcast

                    if it < 3:
                        q_proj_tile(hp, it + 1)
                    if it == 0 and hp < 3:
                        k_proj(hp + 1)
                        q_proj_tile(hp + 1, 0)

            norm_mul(3, 2)
            norm_mul(3, 3)
            oproj(3)

    _split_excess_waits(nc)
    return nc


_CACHED = None


def kernel(x, context, Wq, Wk, Wv, Wo, bo):
    global _CACHED
    if _CACHED is None:
        _CACHED = _build()
    nc = _CACHED

    x = np.asarray(x, dtype=np.float32)
    context = np.asarray(context, dtype=np.float32)
    xf = x.reshape(B, EQ, 64 * 64)
    cf = context.reshape(B, EK, 32 * 32)
    WqT = np.asarray(Wq, np.float32).T.astype(np.float16)   # [EQ, 512]
    WkT = np.asarray(Wk, np.float32).T.astype(np.float16)   # [EK, 512]
    WvT = np.asarray(Wv, np.float32).T.astype(np.float16)   # [EK, 512]
    WoT = np.asarray(Wo, np.float32).T.astype(np.float16)   # [512, OC]
    bo = np.asarray(bo, np.float32)

    # blob1 per batch: ctx chunks + WkT chunks
    def chunks(a, n):
        # [n*128, F] -> [128, n*F] with chunk-major free layout
        return a.reshape(n, 128, -1).transpose(1, 0, 2).reshape(128, -1)

    wk4 = WkT.reshape(4, 128, 4, 128)     # [ec, p, hp, 128]
    wq2 = WqT.reshape(2, 128, 4, 128)     # [ec, p, hp, 128]
    wkt_hp0 = wk4[:, :, 0].transpose(1, 0, 2).reshape(128, 512)
    wqt_hp0 = wq2[:, :, 0].transpose(1, 0, 2).reshape(128, 256)
    wkt_r = wk4[:, :, 1:].transpose(1, 0, 2, 3).reshape(128, 1536)
    wqt_r = wq2[:, :, 1:].transpose(1, 0, 2, 3).reshape(128, 768)
    blobB1 = np.ascontiguousarray(
        np.concatenate([chunks(WvT, 4), wkt_r, wqt_r], axis=1))
    bo2 = np.ascontiguousarray(bo.reshape(2, 128).T)

    in_maps = []
    for core in range(8):
        b, half = core // 2, core % 2
        xh = xf[b, :, half * NQ:(half + 1) * NQ].astype(np.float16).reshape(2, 128, 4, 512)
        x_it0 = xh[:, :, 0].transpose(1, 0, 2).reshape(128, 1024)
        x_r = xh[:, :, 1:].transpose(1, 0, 2, 3).reshape(128, 3072)
        blobA = np.ascontiguousarray(np.concatenate(
            [chunks(cf[b].astype(np.float16), 4), wkt_hp0, wqt_hp0, x_it0], axis=1))
        blobB2 = np.ascontiguousarray(
            np.concatenate([x_r, chunks(WoT, 4)], axis=1))
        in_maps.append({
            "blobA": blobA, "blobB1": blobB1, "blobB2": blobB2, "bo2": bo2,
        })

    res = run_bass_kernel_spmd(nc, in_maps, list(range(8)))
    kernel.last_results = res

    out = np.empty((B, OC, 64 * 64), dtype=np.float32)
    for core in range(8):
        b, half = core // 2, core % 2
        out[b, :, half * NQ:(half + 1) * NQ] = res.results[core]["y"]
    return out.reshape(B, OC, 64, 64)
